# revision 11
# baseline (speedup 1.0000x reference)
"""Trainium2 Bass kernel for nn_CoAttention.

Math: the reference computes additive co-attention where the score matrix
decomposes as an outer sum  scores[l, a] = f(l) + g(a) + c.  Softmax over the
last axis makes the f(l) + c terms cancel exactly, so the attention weights
(and therefore each output row) are independent of l:

    att_audio_features[b, l, :] = softmax_a(tanh(audio[b] @ Wa1.T) @ w_att1[D:]) @ audio[b]
    att_text_features[b, l, :]  = softmax_k(tanh(text[b]  @ Wt2.T) @ w_att2[D:]) @ text[b]

Per batch the device computes the two tanh-projections (the only heavy
matmuls), the weighted score reductions, the softmax normalization, the two
weighted sums, and broadcasts each resulting D-vector to all L output rows.

Sharding: data-parallel over batch, 2 batches per core on 8 cores; weights
replicated.  Host-side prep is layout only: bf16 cast + per-batch transpose
so the contraction dim (d) lands on SBUF partitions.
"""

import os
from contextlib import ExitStack

import ml_dtypes
import numpy as np

B, L, A, D = 16, 1024, 512, 1024
NCORES = 8
BPC = B // NCORES  # batches per core
P = 128  # SBUF partitions
DT = D // P  # d tiles
KT = L // P  # text row tiles
AT = A // P  # audio row tiles

_CACHE = {}
LAST_RESULTS = None


def _ensure_axon_hooks():
    """Some images lack antenv.axon_hooks; provide it + register the NTFF
    profile hook so trace=True works instead of crashing on import."""
    import sys
    import types
    try:
        import antenv.axon_hooks  # noqa: F401
        return
    except ImportError:
        pass
    try:
        import antenv
        mod = types.ModuleType("antenv.axon_hooks")
        _hook = [None]
        mod.set_axon_ntff_profile_hook = lambda h: _hook.__setitem__(0, h)
        mod.get_axon_ntff_profile_hook = lambda: _hook[0]
        sys.modules["antenv.axon_hooks"] = mod
        antenv.axon_hooks = mod
        try:
            from trn_agent_boot.trn_boot import _ntff_profile_via_ctypes
            _hook[0] = _ntff_profile_via_ctypes("/opt/axon/libaxon_pjrt.so")
        except Exception:
            pass
    except Exception:
        pass


def _build_program():
    import concourse.bass as bass
    import concourse.mybir as mybir
    import concourse.tile as tile
    from concourse import bacc

    BF = mybir.dt.bfloat16
    F32 = mybir.dt.float32
    Tanh = mybir.ActivationFunctionType.Tanh
    Exp = mybir.ActivationFunctionType.Exp
    Copy = mybir.ActivationFunctionType.Copy
    mult = mybir.AluOpType.mult
    add = mybir.AluOpType.add

    # Bacc (not plain Bass): its compile() runs generate_event_semaphores,
    # which splits multi-wait sync info — TRN2 instructions allow only one
    # embedded semaphore wait.
    nc = bacc.Bacc("TRN2", target_bir_lowering=False, debug=False, num_devices=NCORES)

    # DRAM I/O (per-core shapes)
    texT = nc.dram_tensor("texT", [BPC, D, L], BF, kind="ExternalInput").ap()
    texN = nc.dram_tensor("texN", [BPC, L, D], BF, kind="ExternalInput").ap()
    audT = nc.dram_tensor("audT", [BPC, D, A], BF, kind="ExternalInput").ap()
    audN = nc.dram_tensor("audN", [BPC, A, D], BF, kind="ExternalInput").ap()
    wa1t = nc.dram_tensor("wa1t", [D, D], BF, kind="ExternalInput").ap()
    wt2t = nc.dram_tensor("wt2t", [D, D], BF, kind="ExternalInput").ap()
    w1b = nc.dram_tensor("w1b", [P, D], BF, kind="ExternalInput").ap()
    w2b = nc.dram_tensor("w2b", [P, D], BF, kind="ExternalInput").ap()
    out_text = nc.dram_tensor("out_text", [BPC, L, D], F32, kind="ExternalOutput").ap()
    out_audio = nc.dram_tensor("out_audio", [BPC, L, D], F32, kind="ExternalOutput").ap()

    with tile.TileContext(nc) as tc, ExitStack() as ctx:
        wpool = ctx.enter_context(tc.tile_pool(name="weights", bufs=1))
        inpool = ctx.enter_context(tc.tile_pool(name="inputs", bufs=2))
        tpool = ctx.enter_context(tc.tile_pool(name="tanh", bufs=3))
        # bufs=4: one fresh slot per (stage, batch) use — avoids WAR sem waits,
        # which matters because DVE instructions only support ONE embedded wait
        # in this walrus build.
        spool = ctx.enter_context(tc.tile_pool(name="small", bufs=4))
        obpool = ctx.enter_context(tc.tile_pool(name="outbuf", bufs=4))
        ypsum = ctx.enter_context(tc.tile_pool(name="ypsum", bufs=2, space="PSUM"))
        opsum = ctx.enter_context(tc.tile_pool(name="opsum", bufs=2, space="PSUM"))

        # Replicated weights: [d, e] with d on partitions, as DT tiles.
        wa1t_sb = wpool.tile([P, DT, D], BF)
        nc.sync.dma_start(out=wa1t_sb[:], in_=wa1t.rearrange("(dt p) e -> p dt e", p=P))
        w1b_sb = wpool.tile([P, D], BF)
        nc.sync.dma_start(out=w1b_sb[:], in_=w1b)
        wt2t_sb = wpool.tile([P, DT, D], BF)
        nc.sync.dma_start(out=wt2t_sb[:], in_=wt2t.rearrange("(dt p) e -> p dt e", p=P))
        w2b_sb = wpool.tile([P, D], BF)
        nc.sync.dma_start(out=w2b_sb[:], in_=w2b)

        ones_col = wpool.tile([P, 1], F32)
        nc.gpsimd.memset(ones_col[:], 1.0)
        ones_row = wpool.tile([1, P], F32)
        nc.gpsimd.memset(ones_row[:], 1.0)

        # DVE instructions support only one embedded sem wait on this walrus
        # build.  Touch the score-vector weights on DVE once so the per-tile
        # multiplies below never need to wait on their DMA sem again.
        wtouch = wpool.tile([1, 2], BF)
        nc.vector.tensor_copy(out=wtouch[0:1, 0:1], in_=w1b_sb[0:1, 0:1])
        nc.vector.tensor_copy(out=wtouch[0:1, 1:2], in_=w2b_sb[0:1, 0:1])

        def attend(rowT_sb, rowN_sb, w_proj_sb, wv_sb, n_rt, out_dram_b):
            """One attention stage: rows [n_rt*128, D] -> output row broadcast.

            rowT_sb: [P, DT, n_rt*128] bf16  (transposed rows: d on partitions)
            rowN_sb: [P, n_rt, D] bf16       (natural rows: row index on partitions)
            w_proj_sb: [P, DT, D] bf16       (projection weight, d on partitions)
            wv_sb: [P, D] bf16               (score vector, replicated across partitions)
            """
            N = n_rt * P
            # scores s[r] = sum_e tanh(sum_d row[r,d] W[e,d]) * wv[e]
            sv = spool.tile([P, n_rt], F32, tag="sv", name="sv")
            for rt in range(n_rt):
                py = ypsum.tile([P, D], F32, tag="y", name="py")
                for dt_ in range(DT):
                    lhs = rowT_sb[:, dt_, rt * P:(rt + 1) * P]
                    nc.tensor.matmul(py[:, 0:512], lhs, w_proj_sb[:, dt_, 0:512],
                                     start=(dt_ == 0), stop=(dt_ == DT - 1))
                    nc.tensor.matmul(py[:, 512:1024], lhs, w_proj_sb[:, dt_, 512:1024],
                                     start=(dt_ == 0), stop=(dt_ == DT - 1))
                th = tpool.tile([P, D], BF, tag="t", name="th")
                nc.scalar.activation(th[:], py[:], Tanh)
                ttr = tpool.tile([P, D], F32, tag="ttr", name="ttr")
                nc.vector.tensor_mul(out=ttr[:], in0=th[:], in1=wv_sb[:])
                nc.vector.reduce_sum(out=sv[:, rt:rt + 1], in_=ttr[:],
                                     axis=mybir.AxisListType.XYZW)
            # softmax numerator (bf16) + per-partition partial sums (fp32)
            ev = spool.tile([P, n_rt], BF, tag="ev", name="ev")
            zp = spool.tile([P, 1], F32, tag="zp", name="zp")
            nc.scalar.activation(ev[:], sv[:], Exp, accum_out=zp[:])
            # total Z = sum over partitions, then 1/Z
            zps = opsum.tile([1, 1], F32, tag="o", name="zps")
            nc.tensor.matmul(zps[:], ones_col[:], zp[:], start=True, stop=True)
            rz = spool.tile([1, 1], F32, tag="rz", name="rz")
            nc.vector.reciprocal(rz[:], zps[:])
            # weighted sum of natural rows: num[d] = sum_r e[r] * row[r, d]
            fps = opsum.tile([1, D], F32, tag="o", name="fps")
            for rt in range(n_rt):
                nc.tensor.matmul(fps[:, 0:512], ev[:, rt:rt + 1], rowN_sb[:, rt, 0:512],
                                 start=(rt == 0), stop=(rt == n_rt - 1))
                nc.tensor.matmul(fps[:, 512:1024], ev[:, rt:rt + 1], rowN_sb[:, rt, 512:1024],
                                 start=(rt == 0), stop=(rt == n_rt - 1))
            att = spool.tile([1, D], F32, tag="att", name="att")
            nc.scalar.activation(att[:], fps[:], Copy, scale=rz[:])
            # broadcast the row to 128 partitions (exact fp32 matmul vs ones)
            bps = opsum.tile([P, D], F32, tag="o", name="bps")
            nc.tensor.matmul(bps[:, 0:512], ones_row[:], att[0:1, 0:512], start=True, stop=True)
            nc.tensor.matmul(bps[:, 512:1024], ones_row[:], att[0:1, 512:1024], start=True, stop=True)
            outb = obpool.tile([P, D], F32, tag="outb", name="outb")
            nc.scalar.copy(outb[:], bps[:])
            # store L rows = 8 blocks of 128 identical rows
            od = out_dram_b.rearrange("(kb p) d -> kb p d", p=P)
            for kb in range(L // P):
                nc.sync.dma_start(out=od[kb], in_=outb[:])

        for b in range(BPC):
            audT_sb = inpool.tile([P, DT, A], BF, tag="audT", name="audT_sb")
            nc.sync.dma_start(out=audT_sb[:], in_=audT[b].rearrange("(dt p) a -> p dt a", p=P))
            audN_sb = inpool.tile([P, AT, D], BF, tag="audN", name="audN_sb")
            nc.sync.dma_start(out=audN_sb[:], in_=audN[b].rearrange("(at p) d -> p at d", p=P))
            texT_sb = inpool.tile([P, DT, L], BF, tag="texT", name="texT_sb")
            nc.sync.dma_start(out=texT_sb[:], in_=texT[b].rearrange("(dt p) k -> p dt k", p=P))
            texN_sb = inpool.tile([P, KT, D], BF, tag="texN", name="texN_sb")
            nc.sync.dma_start(out=texN_sb[:], in_=texN[b].rearrange("(kt p) d -> p kt d", p=P))

            attend(audT_sb, audN_sb, wa1t_sb, w1b_sb, AT, out_audio[b])
            attend(texT_sb, texN_sb, wt2t_sb, w2b_sb, KT, out_text[b])

    nc.compile()
    return nc


def _prep_inputs(text_features, audio_features, Wa1, w_att1, Wt2, w_att2):
    bf16 = ml_dtypes.bfloat16
    tex = np.ascontiguousarray(text_features).astype(bf16)
    aud = np.ascontiguousarray(audio_features).astype(bf16)
    texT = np.ascontiguousarray(tex.transpose(0, 2, 1))
    audT = np.ascontiguousarray(aud.transpose(0, 2, 1))
    wa1t = np.ascontiguousarray(np.asarray(Wa1).T).astype(bf16)
    wt2t = np.ascontiguousarray(np.asarray(Wt2).T).astype(bf16)
    w1b = np.broadcast_to(np.asarray(w_att1)[D:].astype(bf16), (P, D)).copy()
    w2b = np.broadcast_to(np.asarray(w_att2)[D:].astype(bf16), (P, D)).copy()

    in_maps = []
    for c in range(NCORES):
        s = slice(c * BPC, (c + 1) * BPC)
        in_maps.append({
            "texT": texT[s], "texN": tex[s],
            "audT": audT[s], "audN": aud[s],
            "wa1t": wa1t, "wt2t": wt2t, "w1b": w1b, "w2b": w2b,
        })
    return in_maps


def kernel(text_features, audio_features, Wt1, bt1, Wa1, w_att1, b_att1,
           Wt2, Wa2, ba2, w_att2, b_att2):
    global LAST_RESULTS
    _ensure_axon_hooks()
    from concourse.bass_utils import run_bass_kernel_spmd

    if "nc" not in _CACHE:
        _CACHE["nc"] = _build_program()
    nc = _CACHE["nc"]

    in_maps = _prep_inputs(text_features, audio_features, Wa1, w_att1, Wt2, w_att2)
    res = run_bass_kernel_spmd(nc, in_maps, list(range(NCORES)))
    LAST_RESULTS = res

    att_text = np.concatenate([res.results[c]["out_text"] for c in range(NCORES)], axis=0)
    att_audio = np.concatenate([res.results[c]["out_audio"] for c in range(NCORES)], axis=0)
    return att_text, att_audio


# revision 18
# speedup vs baseline: 1.0117x; 1.0117x over previous
"""Trainium2 Bass kernel for nn_CoAttention.

Math: the reference computes additive co-attention where the score matrix
decomposes as an outer sum  scores[l, a] = f(l) + g(a) + c.  Softmax over the
last axis makes the f(l) + c terms cancel exactly, so the attention weights
(and therefore each output row) are independent of l:

    att_audio_features[b, l, :] = softmax_a(tanh(audio[b] @ Wa1.T) @ w_att1[D:]) @ audio[b]
    att_text_features[b, l, :]  = softmax_k(tanh(text[b]  @ Wt2.T) @ w_att2[D:]) @ text[b]

Per batch the device computes the two tanh-projections (the only heavy
matmuls), the weighted score reductions, the softmax normalization, the two
weighted sums, and broadcasts each resulting D-vector to all L output rows.

Sharding: data-parallel over batch, 2 batches per core on 8 cores; weights
replicated.  Host-side prep is layout only: bf16 cast + per-batch transpose
so the contraction dim (d) lands on SBUF partitions.
"""

import os
from contextlib import ExitStack

import ml_dtypes
import numpy as np

B, L, A, D = 16, 1024, 512, 1024
NCORES = 8
BPC = B // NCORES  # batches per core
P = 128  # SBUF partitions
DT = D // P  # d tiles
KT = L // P  # text row tiles
AT = A // P  # audio row tiles

_CACHE = {}
LAST_RESULTS = None


def _ensure_axon_hooks():
    """Some images lack antenv.axon_hooks; provide it + register the NTFF
    profile hook so trace=True works instead of crashing on import."""
    import sys
    import types
    try:
        import antenv.axon_hooks  # noqa: F401
        return
    except ImportError:
        pass
    try:
        import antenv
        mod = types.ModuleType("antenv.axon_hooks")
        _hook = [None]
        mod.set_axon_ntff_profile_hook = lambda h: _hook.__setitem__(0, h)
        mod.get_axon_ntff_profile_hook = lambda: _hook[0]
        sys.modules["antenv.axon_hooks"] = mod
        antenv.axon_hooks = mod
        try:
            from trn_agent_boot.trn_boot import _ntff_profile_via_ctypes
            _hook[0] = _ntff_profile_via_ctypes("/opt/axon/libaxon_pjrt.so")
        except Exception:
            pass
    except Exception:
        pass


def _build_program():
    import concourse.bass as bass
    import concourse.mybir as mybir
    import concourse.tile as tile
    from concourse import bacc

    BF = mybir.dt.bfloat16
    F32 = mybir.dt.float32
    F32R = mybir.dt.float32r
    Tanh = mybir.ActivationFunctionType.Tanh
    Exp = mybir.ActivationFunctionType.Exp
    Copy = mybir.ActivationFunctionType.Copy
    mult = mybir.AluOpType.mult
    add = mybir.AluOpType.add

    # Bacc (not plain Bass): its compile() runs generate_event_semaphores,
    # which splits multi-wait sync info — TRN2 instructions allow only one
    # embedded semaphore wait.
    nc = bacc.Bacc("TRN2", target_bir_lowering=False, debug=False, num_devices=NCORES)

    # DRAM I/O (per-core shapes)
    texT = nc.dram_tensor("texT", [BPC, D, L], BF, kind="ExternalInput").ap()
    texN = nc.dram_tensor("texN", [BPC, L, D], BF, kind="ExternalInput").ap()
    audT = nc.dram_tensor("audT", [BPC, D, A], BF, kind="ExternalInput").ap()
    audN = nc.dram_tensor("audN", [BPC, A, D], BF, kind="ExternalInput").ap()
    wa1t = nc.dram_tensor("wa1t", [D, D], BF, kind="ExternalInput").ap()
    wt2t = nc.dram_tensor("wt2t", [D, D], BF, kind="ExternalInput").ap()
    w1b = nc.dram_tensor("w1b", [P, D], BF, kind="ExternalInput").ap()
    w2b = nc.dram_tensor("w2b", [P, D], BF, kind="ExternalInput").ap()
    out_text = nc.dram_tensor("out_text", [BPC, L, D], F32, kind="ExternalOutput").ap()
    out_audio = nc.dram_tensor("out_audio", [BPC, L, D], F32, kind="ExternalOutput").ap()

    with tile.TileContext(nc) as tc, ExitStack() as ctx:
        wpool = ctx.enter_context(tc.tile_pool(name="weights", bufs=1))
        inpool = ctx.enter_context(tc.tile_pool(name="inputs", bufs=2))
        tpool = ctx.enter_context(tc.tile_pool(name="tanh", bufs=3))
        # bufs=4: one fresh slot per (stage, batch) use — avoids WAR sem waits,
        # which matters because DVE instructions only support ONE embedded wait
        # in this walrus build.
        spool = ctx.enter_context(tc.tile_pool(name="small", bufs=4))
        obpool = ctx.enter_context(tc.tile_pool(name="outbuf", bufs=4))
        ypsum = ctx.enter_context(tc.tile_pool(name="ypsum", bufs=2, space="PSUM"))
        opsum = ctx.enter_context(tc.tile_pool(name="opsum", bufs=2, space="PSUM"))

        # Replicated weights: [d, e] with d on partitions, as DT tiles.
        wa1t_sb = wpool.tile([P, DT, D], BF)
        nc.sync.dma_start(out=wa1t_sb[:], in_=wa1t.rearrange("(dt p) e -> p dt e", p=P))
        w1b_sb = wpool.tile([P, D], BF)
        nc.sync.dma_start(out=w1b_sb[:], in_=w1b)
        wt2t_sb = wpool.tile([P, DT, D], BF)
        nc.sync.dma_start(out=wt2t_sb[:], in_=wt2t.rearrange("(dt p) e -> p dt e", p=P))
        w2b_sb = wpool.tile([P, D], BF)
        nc.sync.dma_start(out=w2b_sb[:], in_=w2b)

        ones_col = wpool.tile([P, 1], F32)
        nc.gpsimd.memset(ones_col[:], 1.0)
        # memset can't write float32r; produce the rounded ones row via ACT
        ones_f32 = wpool.tile([1, P], F32)
        nc.gpsimd.memset(ones_f32[:], 1.0)
        ones_row = wpool.tile([1, P], F32R)
        nc.scalar.copy(ones_row[:], ones_f32[:])

        # DVE instructions support only one embedded sem wait on this walrus
        # build.  Touch the score-vector weights on DVE once so the per-tile
        # multiplies below never need to wait on their DMA sem again.
        wtouch = wpool.tile([1, 2], BF)
        nc.vector.tensor_copy(out=wtouch[0:1, 0:1], in_=w1b_sb[0:1, 0:1])
        nc.vector.tensor_copy(out=wtouch[0:1, 1:2], in_=w2b_sb[0:1, 0:1])

        # PE pre-warm: ~4us of tiny matmuls issued while the first input DMAs
        # are in flight, so the HAM clock gate releases (1.2 -> 2.4 GHz)
        # before the real matmuls start.
        wsb = wpool.tile([P, 1], F32)
        nc.gpsimd.memset(wsb[:], 0.0)
        for _ in range(48):
            wps = opsum.tile([1, 1], F32, tag="o", name="wps")
            nc.tensor.matmul(wps[:], wsb[:], wsb[:], start=True, stop=True)

        def scores(rowT_sb, w_proj_sb, wv_sb, n_rt):
            """Projection + tanh + weighted reduce: sv[r] = sum_e tanh(row W.T)[r,e] wv[e].

            rowT_sb: [P, DT, n_rt*128] bf16  (transposed rows: d on partitions)
            w_proj_sb: [P, DT, D] bf16       (projection weight, d on partitions)
            wv_sb: [P, D] bf16               (score vector, replicated across partitions)
            """
            sv = spool.tile([P, n_rt], F32, tag="sv", name="sv")
            for rt in range(n_rt):
                py = ypsum.tile([P, D], F32, tag="y", name="py")
                for dt_ in range(DT):
                    lhs = rowT_sb[:, dt_, rt * P:(rt + 1) * P]
                    nc.tensor.matmul(py[:, 0:512], lhs, w_proj_sb[:, dt_, 0:512],
                                     start=(dt_ == 0), stop=(dt_ == DT - 1))
                    nc.tensor.matmul(py[:, 512:1024], lhs, w_proj_sb[:, dt_, 512:1024],
                                     start=(dt_ == 0), stop=(dt_ == DT - 1))
                th = tpool.tile([P, D], BF, tag="t", name="th")
                nc.scalar.activation(th[:], py[:], Tanh)
                ttr = tpool.tile([P, D], F32, tag="ttr", name="ttr")
                nc.vector.tensor_mul(out=ttr[:], in0=th[:], in1=wv_sb[:])
                nc.vector.reduce_sum(out=sv[:, rt:rt + 1], in_=ttr[:],
                                     axis=mybir.AxisListType.X)
            # softmax numerator (bf16) + per-partition partial sums (fp32)
            ev = spool.tile([P, n_rt], BF, tag="ev", name="ev")
            zp = spool.tile([P, 1], F32, tag="zp", name="zp")
            nc.scalar.activation(ev[:], sv[:], Exp, accum_out=zp[:])
            return ev, zp

        def finalize(ev, zp, rowN_sb, n_rt, out_dram_b):
            """softmax normalize + weighted row sum + broadcast to L rows."""
            # total Z = sum over partitions, then 1/Z
            zps = opsum.tile([1, 1], F32, tag="o", name="zps")
            nc.tensor.matmul(zps[:], ones_col[:], zp[:], start=True, stop=True)
            rz = spool.tile([1, 1], F32, tag="rz", name="rz")
            nc.vector.reciprocal(rz[:], zps[:])
            # weighted sum of natural rows: num[d] = sum_r e[r] * row[r, d]
            fps = opsum.tile([1, D], F32, tag="o", name="fps")
            for rt in range(n_rt):
                nc.tensor.matmul(fps[:, 0:512], ev[:, rt:rt + 1], rowN_sb[:, rt, 0:512],
                                 start=(rt == 0), stop=(rt == n_rt - 1))
                nc.tensor.matmul(fps[:, 512:1024], ev[:, rt:rt + 1], rowN_sb[:, rt, 512:1024],
                                 start=(rt == 0), stop=(rt == n_rt - 1))
            att = spool.tile([1, D], F32R, tag="att", name="att")
            nc.scalar.activation(att[:], fps[:], Copy, scale=rz[:])
            # broadcast the row to 128 partitions; fp32r streams at bf16 rate
            # for N>=256 and the K=1 "sum" is exact (single product by 1.0)
            bps = opsum.tile([P, D], F32, tag="o", name="bps")
            nc.tensor.matmul(bps[:, 0:512], ones_row[:], att[0:1, 0:512], start=True, stop=True)
            nc.tensor.matmul(bps[:, 512:1024], ones_row[:], att[0:1, 512:1024], start=True, stop=True)
            outb = obpool.tile([P, D], F32, tag="outb", name="outb")
            nc.scalar.copy(outb[:], bps[:])
            # store L rows = 8 blocks of 128 identical rows
            od = out_dram_b.rearrange("(kb p) d -> kb p d", p=P)
            for kb in range(L // P):
                nc.sync.dma_start(out=od[kb], in_=outb[:])

        # Load both batches' inputs up front (DMA queues drain under compute).
        ins = []
        for b in range(BPC):
            audT_sb = inpool.tile([P, DT, A], BF, tag="audT", name="audT_sb")
            nc.sync.dma_start(out=audT_sb[:], in_=audT[b].rearrange("(dt p) a -> p dt a", p=P))
            audN_sb = inpool.tile([P, AT, D], BF, tag="audN", name="audN_sb")
            nc.sync.dma_start(out=audN_sb[:], in_=audN[b].rearrange("(at p) d -> p at d", p=P))
            texT_sb = inpool.tile([P, DT, L], BF, tag="texT", name="texT_sb")
            nc.sync.dma_start(out=texT_sb[:], in_=texT[b].rearrange("(dt p) k -> p dt k", p=P))
            texN_sb = inpool.tile([P, KT, D], BF, tag="texN", name="texN_sb")
            nc.sync.dma_start(out=texN_sb[:], in_=texN[b].rearrange("(kt p) d -> p kt d", p=P))
            ins.append((audT_sb, audN_sb, texT_sb, texN_sb))

        # Schedule: all heavy score matmuls first (A0 T0 A1), then batch-0
        # finals (whose softmax inputs are long since ready -> no PE stall),
        # then T1, then batch-1 finals.  Output DMAs of F0 overlap T1.
        ev_a0, zp_a0 = scores(ins[0][0], wa1t_sb, w1b_sb, AT)
        ev_t0, zp_t0 = scores(ins[0][2], wt2t_sb, w2b_sb, KT)
        ev_a1, zp_a1 = scores(ins[1][0], wa1t_sb, w1b_sb, AT)
        finalize(ev_a0, zp_a0, ins[0][1], AT, out_audio[0])
        finalize(ev_t0, zp_t0, ins[0][3], KT, out_text[0])
        ev_t1, zp_t1 = scores(ins[1][2], wt2t_sb, w2b_sb, KT)
        finalize(ev_a1, zp_a1, ins[1][1], AT, out_audio[1])
        finalize(ev_t1, zp_t1, ins[1][3], KT, out_text[1])

    nc.compile()
    return nc


def _prep_inputs(text_features, audio_features, Wa1, w_att1, Wt2, w_att2):
    bf16 = ml_dtypes.bfloat16
    tex = np.ascontiguousarray(text_features).astype(bf16)
    aud = np.ascontiguousarray(audio_features).astype(bf16)
    texT = np.ascontiguousarray(tex.transpose(0, 2, 1))
    audT = np.ascontiguousarray(aud.transpose(0, 2, 1))
    wa1t = np.ascontiguousarray(np.asarray(Wa1).T).astype(bf16)
    wt2t = np.ascontiguousarray(np.asarray(Wt2).T).astype(bf16)
    w1b = np.broadcast_to(np.asarray(w_att1)[D:].astype(bf16), (P, D)).copy()
    w2b = np.broadcast_to(np.asarray(w_att2)[D:].astype(bf16), (P, D)).copy()

    in_maps = []
    for c in range(NCORES):
        s = slice(c * BPC, (c + 1) * BPC)
        in_maps.append({
            "texT": texT[s], "texN": tex[s],
            "audT": audT[s], "audN": aud[s],
            "wa1t": wa1t, "wt2t": wt2t, "w1b": w1b, "w2b": w2b,
        })
    return in_maps


def kernel(text_features, audio_features, Wt1, bt1, Wa1, w_att1, b_att1,
           Wt2, Wa2, ba2, w_att2, b_att2):
    global LAST_RESULTS
    _ensure_axon_hooks()
    from concourse.bass_utils import run_bass_kernel_spmd

    if "nc" not in _CACHE:
        _CACHE["nc"] = _build_program()
    nc = _CACHE["nc"]

    in_maps = _prep_inputs(text_features, audio_features, Wa1, w_att1, Wt2, w_att2)
    res = run_bass_kernel_spmd(nc, in_maps, list(range(NCORES)))
    LAST_RESULTS = res

    att_text = np.concatenate([res.results[c]["out_text"] for c in range(NCORES)], axis=0)
    att_audio = np.concatenate([res.results[c]["out_audio"] for c in range(NCORES)], axis=0)
    return att_text, att_audio
